# revision 13
# baseline (speedup 1.0000x reference)
"""Multi-head attention Bass/Tile kernel for TRN2, sharded 8 ways.

Sharding: core c handles batch b = c//2 and heads half = c%2 (8 of 16 heads).
Each core computes, for its batch and its 8 heads:
  q/k/v projections -> scoresT = K @ Q^T (per head, [t, s] layout) -> exp ->
  PV matmul with a ones-column appended to V (gives row sums for free) ->
  normalize -> partial output projection against its 512 rows of Wout^T.
Host sums the two partials per batch and adds the bias.

Layout choices (all chosen so NO transposes are needed anywhere):
  xT     [D, S]  : host-pretransposed activations (d on partitions)
  wq/wk  [D, H*dk] : lhsT layout for qT/kT = W^T @ xT
  wv     [D, H*dk] : rhs layout for v = xT^T @ wv  ([t, vdim], natural)
  kT     [H*dk, S]: j on partitions -> head-pair p lives in 128-row chunk p
                    (head A in rows 0:64, head B in rows 64:128)
  qT     [128, S_BLK] per (hp, sb): head A q-dims in rows 0:64, B in 64:128
  scoresT[t, s]   : ROW-TILED pair of K=64 matmuls — head A contracts on PE
                    array rows 0-63 (tile_position (0,0)), head B on rows
                    64-127 ((64,0), auto-derived from base_partition 64 of
                    kT[64:128]/qT[64:128]).  The two MMs run CONCURRENTLY
                    (different row groups, different psum banks) — ~2x
                    scores throughput vs a zero-padded K=128 formulation,
                    and no memsets/padded copies.
  ps tile [128, 1024] f32 (2 banks) = {A-chunk | B-chunk}: one exp
                    activation instr (N=1024) per t-chunk covers both heads.
  softmax sum over t folded into the PV matmul via the ones column of v'.
  out    [s, o]   : lhsT=concatT [i,s], rhs=woutT [i,o]

Scheduling: one software pipeline over units (sb, hp).  The ACT engine is
the bottleneck (~286us of exp work), so the prologue before the first
scores matmul is minimal (kT tb0/jc0 + qT0, 40 MMs): everything else (kT
remainder, v' projection, later qT blocks, out-projections) rides as
fillers inside the scores loops.  PV of unit k-1 trails unit k's scores
by 2 chunks.  Emission order is dependency-consistent per engine queue
(producer MMs always emitted before consumer MMs): kT jc0/jc1 complete
within unit 0's fillers, v' chunks complete ahead of their PV consumers
in unit 1, xk blocks stay resident (bufs=4) to avoid reload ordering.

HW pitfalls baked in (learned on-device):
  - no partition-shifting DVE copies (sim allows them, HW corrupts);
    the only cross-partition moves are InstReciprocal sbuf[64:65]->sbuf[0:1]
    (verified on HW) and gpsimd partition_broadcast
  - reciprocal_approx_fast (custom DVE op) produces garbage on HW
  - matmul free dim capped at 512 (fp32 psum); psum tiles bank-aligned
"""

from contextlib import ExitStack
from dataclasses import dataclass

import numpy as np
import ml_dtypes

import concourse.bass as bass  # noqa: F401
import concourse.tile as tile
from concourse import bacc, mybir


@dataclass
class Cfg:
    D: int = 1024      # model dim
    S: int = 2048      # sequence length (queries == keys)
    HL: int = 8        # heads per core
    DK: int = 64       # head dim
    S_BLK: int = 512   # query block (matmul free dim)
    T_BLK: int = 512   # t block in projection phase

    @property
    def DC(self):
        return self.D // 128

    @property
    def NSB(self):
        return self.S // self.S_BLK

    @property
    def TBn(self):
        return self.S // self.T_BLK

    @property
    def TCn(self):
        return self.S // 128

    @property
    def JW(self):
        return self.HL * self.DK

    @property
    def JC(self):
        return self.JW // 128

    @property
    def VW(self):
        return self.DK + 1

    @property
    def OB(self):
        return min(512, self.D)


DT_NP = {
    mybir.dt.bfloat16: ml_dtypes.bfloat16,
    mybir.dt.float32: np.float32,
    mybir.dt.float32r: np.float32,
}


def build_nc(cfg: Cfg, DT=mybir.dt.bfloat16, num_devices: int = 8):
    c = cfg
    f32 = mybir.dt.float32
    EXPDT = DT if DT == mybir.dt.bfloat16 else f32
    SCALE = 1.0 / float(np.sqrt(c.DK))
    nc = bacc.Bacc("TRN2", target_bir_lowering=False, debug=False,
                   num_devices=num_devices)

    xqT = nc.dram_tensor("xqT", [c.D, c.S], DT, kind="ExternalInput").ap()
    xkT = nc.dram_tensor("xkT", [c.D, c.S], DT, kind="ExternalInput").ap()
    xvT = nc.dram_tensor("xvT", [c.D, c.S], DT, kind="ExternalInput").ap()
    wq_d = nc.dram_tensor("wq", [c.D, c.JW], DT, kind="ExternalInput").ap()
    wk_d = nc.dram_tensor("wk", [c.D, c.JW], DT, kind="ExternalInput").ap()
    wv_d = nc.dram_tensor("wv", [c.D, c.JW], DT, kind="ExternalInput").ap()
    wo_d = nc.dram_tensor("woutT", [c.JW, c.D], DT, kind="ExternalInput").ap()
    out_d = nc.dram_tensor("out", [c.S, c.D], f32, kind="ExternalOutput").ap()

    with tile.TileContext(nc) as tc, ExitStack() as es:
        wpool = es.enter_context(tc.tile_pool(name="weights", bufs=1))
        kvpool = es.enter_context(tc.tile_pool(name="kv", bufs=1))
        xpool = es.enter_context(tc.tile_pool(name="x", bufs=2))
        qpool = es.enter_context(tc.tile_pool(name="q", bufs=2))
        epool = es.enter_context(tc.tile_pool(name="exp", bufs=2))
        cpool = es.enter_context(tc.tile_pool(name="cat", bufs=2))
        opool = es.enter_context(tc.tile_pool(name="o", bufs=2))
        rpool = es.enter_context(tc.tile_pool(name="r", bufs=1))
        pspool = es.enter_context(tc.tile_pool(name="ps", bufs=2, space="PSUM"))
        pvpool = es.enter_context(tc.tile_pool(name="pv", bufs=2, space="PSUM"))
        fppool = es.enter_context(tc.tile_pool(name="fp", bufs=2, space="PSUM"))
        stpool = es.enter_context(tc.tile_pool(name="st", bufs=2))

        def load_w_dmaj(dram, tag):
            t = wpool.tile([128, c.DC * c.JW], DT, tag=tag, name=tag)
            for d in range(c.DC):
                eng = nc.sync if d % 2 == 0 else nc.gpsimd
                eng.dma_start(t[:, d * c.JW:(d + 1) * c.JW],
                              dram[d * 128:(d + 1) * 128, :])
            return t

        def load_x_blk(dram, blk, name, tag="x", bufs=None):
            t = xpool.tile([128, c.DC * c.S_BLK], DT, tag=tag, name=name,
                           bufs=bufs)
            for d in range(c.DC):
                eng = nc.sync if d % 2 == 0 else nc.gpsimd
                eng.dma_start(
                    t[:, d * c.S_BLK:(d + 1) * c.S_BLK],
                    dram[d * 128:(d + 1) * 128,
                         blk * c.S_BLK:(blk + 1) * c.S_BLK])
            return t

        kT_sb = kvpool.tile([128, c.JC * c.S], DT)
        v_sb = kvpool.tile([128, c.TCn * c.HL * c.VW], DT)
        box = {}  # late-bound tiles: xk/xv/wv/wo

        # ---- leading weight + activation DMAs ----
        wk_sb = load_w_dmaj(wk_d, "wk")
        box["xk0"] = load_x_blk(xkT, 0, "xk0", tag="xk", bufs=4)
        wq_sb = load_w_dmaj(wq_d, "wq")
        xq_tiles = {0: load_x_blk(xqT, 0, "xq0")}

        def kt_ops(tb, jc):
            """8 accumulating MMs + 1 cast into kT (one jc, one t-block)."""
            ops = []
            pbox = {}

            def mk(d):
                def op():
                    if d == 0:
                        pbox[0] = fppool.tile([128, c.S_BLK], f32, tag="fp",
                                              name=f"psk{tb}_{jc}")
                    nc.tensor.matmul(
                        pbox[0][:],
                        wk_sb[:, d * c.JW + jc * 128: d * c.JW + (jc + 1) * 128],
                        box[f"xk{tb}"][:, d * c.S_BLK:(d + 1) * c.S_BLK],
                        start=(d == 0), stop=(d == c.DC - 1))
                    if d == c.DC - 1:
                        nc.vector.tensor_copy(
                            kT_sb[:, jc * c.S + tb * c.S_BLK:
                                  jc * c.S + (tb + 1) * c.S_BLK],
                            pbox[0][:])
                return op
            for d in range(c.DC):
                ops.append(mk(d))
            return ops

        def qT_ops(sb, qT):
            """Per jc: 8 accumulating MMs + 1 copy (A rows 0:64, B rows
            64:128 — natural psq layout, no zero-pad)."""
            ops = []
            pbox = {}

            def mk(jc, d):
                def op():
                    if d == 0:
                        pbox[jc] = fppool.tile([128, c.S_BLK], f32, tag="fp",
                                               name=f"psq{sb}_{jc}")
                    nc.tensor.matmul(
                        pbox[jc][:],
                        wq_sb[:, d * c.JW + jc * 128: d * c.JW + (jc + 1) * 128],
                        xq_tiles[sb][:, d * c.S_BLK:(d + 1) * c.S_BLK],
                        start=(d == 0), stop=(d == c.DC - 1))
                    if d == c.DC - 1:
                        nc.vector.tensor_copy(
                            qT[:, jc * c.S_BLK:(jc + 1) * c.S_BLK],
                            pbox[jc][:])
                return op
            for jc in range(c.JC):
                for d in range(c.DC):
                    ops.append(mk(jc, d))
            return ops

        def v_ops():
            """Per t-chunk g: 8 accumulating MMs + 1 strided copy into v'.
            xv2/xv3 loads woven in after their predecessors' consumers."""
            ops = []
            pbox = {}

            def mk(g, d):
                tb, tt = divmod(g, c.S_BLK // 128)

                def op():
                    if d == 0:
                        pbox[g] = fppool.tile([128, c.JW], f32, tag="fp",
                                              name=f"psv{g}")
                    nc.tensor.matmul(
                        pbox[g][:],
                        box[f"xv{tb}"][:, d * c.S_BLK + tt * 128:
                                       d * c.S_BLK + (tt + 1) * 128],
                        box["wv"][:, d * c.JW:(d + 1) * c.JW],
                        start=(d == 0), stop=(d == c.DC - 1))
                    if d == c.DC - 1:
                        dst = v_sb[:, g * c.HL * c.VW:(g + 1) * c.HL * c.VW]
                        dst3 = dst.rearrange("p (h w) -> p h w",
                                             w=c.VW)[:, :, 0:c.DK]
                        src3 = pbox[g][:].rearrange("p (h w) -> p h w", w=c.DK)
                        nc.vector.tensor_copy(dst3, src3)
                        del pbox[g]
                return op

            def ld(tb):
                def op():
                    box[f"xv{tb}"] = load_x_blk(xvT, tb, f"xv{tb}")
                return op
            for g in range(c.TCn):
                for d in range(c.DC):
                    ops.append(mk(g, d))
                if g == 3:
                    ops.append(ld(2))
                if g == 7:
                    ops.append(ld(3))
            return ops

        def outproj_ops(sb, catT):
            """Per (sc, oc): 4 ic-MMs into a 1-bank psum, then copy + DMA."""
            ops = []
            po_box = {}

            def mk(sc, oc, ic):
                def op():
                    if ic == 0:
                        po_box[(sc, oc)] = fppool.tile(
                            [128, c.OB], f32, tag="fp", name=f"po{sb}_{sc}_{oc}")
                    po = po_box[(sc, oc)]
                    nc.tensor.matmul(
                        po[:],
                        catT[:, ic * c.S_BLK + sc * 128:
                             ic * c.S_BLK + (sc + 1) * 128],
                        box["wo"][:, ic * c.D + oc * c.OB:
                                  ic * c.D + (oc + 1) * c.OB],
                        start=(ic == 0), stop=(ic == c.JC - 1))
                    if ic == c.JC - 1:
                        ot = opool.tile([128, c.OB], f32, tag="ot",
                                        name=f"ot{sb}_{sc}_{oc}")
                        nc.vector.tensor_copy(ot[:], po[:])
                        eng = nc.sync if (2 * sc + oc) % 2 == 0 else nc.gpsimd
                        eng.dma_start(
                            out_d[sb * c.S_BLK + sc * 128:
                                  sb * c.S_BLK + (sc + 1) * 128,
                                  oc * c.OB:(oc + 1) * c.OB],
                            ot[:])
                return op
            for sc in range(c.S_BLK // 128):
                for oc in range(c.D // c.OB):
                    for ic in range(c.JC):
                        ops.append(mk(sc, oc, ic))
            return ops

        def emit_pv_chunk(u, t0, nt):
            eb = u["exp"]
            for t in range(t0, t0 + nt):
                nc.tensor.matmul(
                    u["pvA"][0:c.VW, :],
                    v_sb[:, t * c.HL * c.VW + (2 * u["hp"]) * c.VW:
                         t * c.HL * c.VW + (2 * u["hp"] + 1) * c.VW],
                    eb[:, t * 1024: t * 1024 + c.S_BLK],
                    start=(t == 0), stop=(t == c.TCn - 1))
                nc.tensor.matmul(
                    u["pvB"][0:c.VW, :],
                    v_sb[:, t * c.HL * c.VW + (2 * u["hp"] + 1) * c.VW:
                         t * c.HL * c.VW + (2 * u["hp"] + 2) * c.VW],
                    eb[:, t * 1024 + c.S_BLK:(t + 1) * 1024],
                    start=(t == 0), stop=(t == c.TCn - 1))

        def emit_stage(u):
            sb, hp = u["sb"], u["hp"]
            u["stA"] = stpool.tile([c.VW, c.S_BLK], f32, tag="stA",
                                   name=f"stA{sb}_{hp}")
            u["stB"] = stpool.tile([c.VW, c.S_BLK], f32, tag="stB",
                                   name=f"stB{sb}_{hp}")
            nc.vector.tensor_copy(u["stA"][:], u["pvA"][0:c.VW, :])
            nc.vector.tensor_copy(u["stB"][:], u["pvB"][0:c.VW, :])

        def emit_normalize(u):
            sb, hp = u["sb"], u["hp"]
            stA, stB, catT = u["stA"], u["stB"], u["catT"]
            rtiA = rpool.tile([1, c.S_BLK], f32, tag="rtiA", name=f"rtiA{sb}_{hp}")
            rtiB = rpool.tile([1, c.S_BLK], f32, tag="rtiB", name=f"rtiB{sb}_{hp}")
            # NB: cross-partition (row 64 -> row 0) — verified OK on HW for
            # InstReciprocal specifically.
            nc.vector.reciprocal(rtiA[:], stA[c.DK:c.DK + 1, :])
            nc.vector.reciprocal(rtiB[:], stB[c.DK:c.DK + 1, :])
            rbA = rpool.tile([c.DK, c.S_BLK], f32, tag="rbA", name=f"rbA{sb}_{hp}")
            rbB = rpool.tile([c.DK, c.S_BLK], f32, tag="rbB", name=f"rbB{sb}_{hp}")
            nc.gpsimd.partition_broadcast(rbA[:], rtiA[:])
            nc.gpsimd.partition_broadcast(rbB[:], rtiB[:])
            nc.vector.tensor_mul(
                catT[0:c.DK, hp * c.S_BLK:(hp + 1) * c.S_BLK],
                stA[0:c.DK, :], rbA[:])
            nc.vector.tensor_mul(
                catT[64:64 + c.DK, hp * c.S_BLK:(hp + 1) * c.S_BLK],
                stB[0:c.DK, :], rbB[:])

        # ---- minimal prologue: kT(tb0, jc0) + qT0 ----
        for op in kt_ops(0, 0):
            op()
        qT_tiles = {0: qpool.tile([128, c.JC * c.S_BLK], DT, tag="qT",
                                  name="qT0")}
        for op in qT_ops(0, qT_tiles[0]):
            op()

        # ---- filler / load schedule ----
        units = [(sb, hp) for sb in range(c.NSB) for hp in range(c.JC)]
        fillers = [[] for _ in units]
        loads = [[] for _ in units]

        def mk_ld(key, fn):
            def op():
                box[key] = fn()
            return op

        # unit 0 loads: remaining xk blocks (resident, bufs=4), wv, first
        # xv blocks; the v' ones-columns memset rides the gpsimd queue last.
        loads[0] += [
            mk_ld("xk1", lambda: load_x_blk(xkT, 1, "xk1", tag="xk", bufs=4)),
            mk_ld("xk2", lambda: load_x_blk(xkT, 2, "xk2", tag="xk", bufs=4)),
            mk_ld("xk3", lambda: load_x_blk(xkT, 3, "xk3", tag="xk", bufs=4)),
            mk_ld("wv", lambda: load_w_dmaj(wv_d, "wv")),
            mk_ld("xv0", lambda: load_x_blk(xvT, 0, "xv0")),
            mk_ld("xv1", lambda: load_x_blk(xvT, 1, "xv1")),
            lambda: nc.gpsimd.memset(v_sb[:], 1.0),
        ]
        def load_wo():
            # wo shares wk's buffer (tag "wk"): its DMA waits on the kT MMs.
            t = wpool.tile([128, c.JC * c.D], DT, tag="wk", name="wo")
            for ic in range(c.JC):
                eng = nc.sync if ic % 2 == 0 else nc.gpsimd
                eng.dma_start(t[:, ic * c.D:(ic + 1) * c.D],
                              wo_d[ic * 128:(ic + 1) * 128, :])
            return t

        loads[1] += [mk_ld("xq1", lambda: load_x_blk(xqT, 1, "xq1"))]
        loads[2] += [mk_ld("wo", load_wo)]

        # unit 0 fillers: finish kT jc0 (scores deadline th4/8/12) and all
        # of jc1 (unit 1's scores), plus tb0's jc2/jc3.
        for tb in [1, 2, 3]:
            fillers[0] += kt_ops(tb, 0)
        for tb in [0, 1, 2, 3]:
            fillers[0] += kt_ops(tb, 1)
        fillers[0] += kt_ops(0, 2)
        fillers[0] += kt_ops(0, 3)          # 72 ops
        # unit 1 fillers: full v' stream (ahead of its PV consumers), then
        # the kT remainder (jc2 by unit 2, jc3 by unit 3).
        fillers[1] += v_ops()               # 130 ops incl 2 loads
        for tb in [1, 2, 3]:
            fillers[1] += kt_ops(tb, 2)
        for tb in [1, 2, 3]:
            fillers[1] += kt_ops(tb, 3)     # +48

        # ---- the main (sb, hp) software pipeline ----
        prev = None
        cat_tiles = {}

        for idx, (sb, hp) in enumerate(units):
            if hp == 0:
                cat_tiles[sb] = cpool.tile([128, c.JC * c.S_BLK], DT, tag="cat",
                                           name=f"catT{sb}")
            for ld in loads[idx]:
                ld()
            if hp == 2 and sb + 2 < c.NSB:
                # prefetch xq for the sb-after-next (qT rides (sb+1,0..1))
                xq_tiles[sb + 2] = load_x_blk(xqT, sb + 2, f"xq{sb + 2}")
            if (sb + 1 < c.NSB and hp == 2 and sb == 0) or \
                    (sb + 1 < c.NSB and hp == 0 and sb >= 1):
                # qT projection for the next sb: units (0,2)-(0,3) for sb1
                # (units 0-1 carry kT/v), else (sb,0)-(sb,1)
                if sb == 0:
                    xq_tiles[1] = box["xq1"]
                qT_tiles[sb + 1] = qpool.tile([128, c.JC * c.S_BLK], DT,
                                              tag="qT", name=f"qT{sb + 1}")
                qops = qT_ops(sb + 1, qT_tiles[sb + 1])
                fillers[idx] += qops[:16]
                fillers[idx + 1] += qops[16:]

            catT = cat_tiles[sb]
            qT = qT_tiles[sb]
            cur = {
                "sb": sb, "hp": hp, "catT": catT,
                "exp": epool.tile([128, c.TCn * 1024], EXPDT, tag="exp",
                                  name=f"exp{sb}_{hp}"),
            }
            if prev is not None:
                prev["pvA"] = pvpool.tile([c.VW, c.S_BLK], f32, tag="pv",
                                          name=f"pvA{prev['sb']}_{prev['hp']}")
                prev["pvB"] = pvpool.tile([c.VW, c.S_BLK], f32, tag="pv",
                                          name=f"pvB{prev['sb']}_{prev['hp']}")
            if idx == len(units) - 1:
                # the last unit's PV rides its own scores loop, using the
                # filler psum (unit 15 carries no fillers) — shrinks the tail
                cur["pvA"] = fppool.tile([c.VW, c.S_BLK], f32, tag="fp",
                                         name="pvA_last")
                cur["pvB"] = fppool.tile([c.VW, c.S_BLK], f32, tag="fp",
                                         name="pvB_last")
            flist = fillers[idx]
            fpos = 0
            # Light-filler units run prev's PV at 2 chunks/th (th2..9) and
            # normalize at th9 — the reciprocal chain then overlaps the
            # unit's second half instead of piling up at unit boundaries.
            # Heavy prologue units (0-1) keep the 1-chunk trail so the v'
            # filler stream stays ahead of its PV consumers.
            front = prev is not None and len(flist) < 48
            for th in range(c.TCn):
                ps = pspool.tile([128, 1024], f32, tag="ps",
                                 name=f"ps{sb}_{hp}_{th}")
                lhsA = kT_sb[0:64, hp * c.S + th * 128: hp * c.S + (th + 1) * 128]
                lhsB = kT_sb[64:128, hp * c.S + th * 128: hp * c.S + (th + 1) * 128]
                # row-tiled concurrent pair: head A on PE rows 0-63,
                # head B on rows 64-127 (tile_position from base_partition)
                nc.tensor.matmul(
                    ps[:, 0:c.S_BLK], lhsA,
                    qT[0:64, hp * c.S_BLK:(hp + 1) * c.S_BLK],
                    start=True, stop=True)
                nc.tensor.matmul(
                    ps[:, c.S_BLK:1024], lhsB,
                    qT[64:128, hp * c.S_BLK:(hp + 1) * c.S_BLK],
                    start=True, stop=True)
                nc.scalar.activation(
                    cur["exp"][:, th * 1024:(th + 1) * 1024],
                    ps[:], mybir.ActivationFunctionType.Exp, scale=SCALE)
                if front:
                    if 2 <= th <= 9:
                        emit_pv_chunk(prev, 2 * (th - 2), 2)
                    if th == 9:
                        emit_stage(prev)
                        emit_normalize(prev)
                elif prev is not None:
                    if th >= 2:
                        emit_pv_chunk(prev, th - 2, 1)
                    if th == c.TCn - 1:
                        emit_pv_chunk(prev, c.TCn - 2, 2)
                        emit_stage(prev)
                if idx == len(units) - 1:
                    if th >= 2:
                        emit_pv_chunk(cur, th - 2, 1)
                    if th == c.TCn - 1:
                        emit_pv_chunk(cur, c.TCn - 2, 2)
                        emit_stage(cur)
                # fillers: back-loaded (th8-15) when PV is front-loaded,
                # else spread evenly
                if front:
                    want = (len(flist) * max(0, th - 7)) // (c.TCn - 8)
                else:
                    want = (len(flist) * (th + 1)) // c.TCn
                while fpos < want:
                    flist[fpos]()
                    fpos += 1
            while fpos < len(flist):
                flist[fpos]()
                fpos += 1
            if prev is not None and not front:
                emit_normalize(prev)
            if prev is not None and prev["hp"] == c.JC - 1:
                # out-projection of prev's sb rides units (sb,2)-(sb,3)
                oops = outproj_ops(prev["sb"], prev["catT"])
                base = idx + 2 if sb + 1 < c.NSB else idx + 1
                fillers[base] += oops[:16]
                if base + 1 < len(units):
                    fillers[base + 1] += oops[16:]
                else:
                    fillers[base] += oops[16:]
            prev = cur
        # ---- drain: the last unit's PV/stage ran in-loop; only its
        # normalize + out-projection remain.  The out-projection's ic=0..2
        # matmuls have no dependency on the last normalize (subtile deps)
        # and fill the PE during the reciprocal/broadcast chain.
        emit_normalize(prev)
        for op in outproj_ops(prev["sb"], prev["catT"]):
            op()

    nc.compile()
    return nc


def shard_inputs(inputs: dict, cfg: Cfg, DT=mybir.dt.bfloat16):
    """Full inputs -> list of 8 per-core in_maps (numpy)."""
    npdt = DT_NP[DT]
    q, k, v = inputs["queries"], inputs["keys"], inputs["values"]
    Wq, Wk, Wv = inputs["Wq"], inputs["Wk"], inputs["Wv"]
    Wout = inputs["Wout"]
    B = q.shape[0]
    maps = []
    WoutT = np.ascontiguousarray(Wout.T)  # [i, o]
    for core in range(2 * B):
        b, half = divmod(core, 2)
        hs = slice(half * cfg.HL, (half + 1) * cfg.HL)
        i0 = half * cfg.JW
        maps.append({
            "xqT": np.ascontiguousarray(q[b].T).astype(npdt),
            "xkT": np.ascontiguousarray(k[b].T).astype(npdt),
            "xvT": np.ascontiguousarray(v[b].T).astype(npdt),
            "wq": np.ascontiguousarray(
                Wq[hs].transpose(1, 0, 2).reshape(cfg.D, cfg.JW)).astype(npdt),
            "wk": np.ascontiguousarray(
                Wk[hs].transpose(1, 0, 2).reshape(cfg.D, cfg.JW)).astype(npdt),
            "wv": np.ascontiguousarray(
                Wv[hs].transpose(1, 0, 2).reshape(cfg.D, cfg.JW)).astype(npdt),
            "woutT": np.ascontiguousarray(WoutT[i0:i0 + cfg.JW]).astype(npdt),
        })
    return maps


def gather_outputs(results, inputs):
    bout = inputs["bout"]
    B = inputs["queries"].shape[0]
    outs = []
    for b in range(B):
        outs.append(results[2 * b]["out"] + results[2 * b + 1]["out"] + bout)
    return np.stack(outs).astype(np.float32)


def percore_reference(in_map: dict, cfg: Cfg):
    """Numpy reference of what one core should produce (fp32 math)."""
    c = cfg
    xq = in_map["xqT"].astype(np.float32).T   # [S, D]
    xk = in_map["xkT"].astype(np.float32).T
    xv = in_map["xvT"].astype(np.float32).T
    wq = in_map["wq"].astype(np.float32)      # [D, JW]
    wk = in_map["wk"].astype(np.float32)
    wv = in_map["wv"].astype(np.float32)
    wo = in_map["woutT"].astype(np.float32)   # [JW, D]
    q = xq @ wq                               # [S, JW]
    k = xk @ wk
    v = xv @ wv
    cat = np.zeros((c.S, c.JW), dtype=np.float32)
    for h in range(c.HL):
        sl = slice(h * c.DK, (h + 1) * c.DK)
        s = (q[:, sl] @ k[:, sl].T) / np.sqrt(c.DK)
        e = np.exp(s)
        p = e / e.sum(axis=1, keepdims=True)
        cat[:, sl] = p @ v[:, sl]
    return cat @ wo

# ----------------------------------------------------------------------------
# Self-contained entry point: kernel(**inputs) -> full [B, S, D] output.
# ----------------------------------------------------------------------------
_NC_CACHE = {}


def _get_nc():
    key = "attn"
    if key not in _NC_CACHE:
        _NC_CACHE[key] = build_nc(Cfg(), mybir.dt.bfloat16, num_devices=8)
    return _NC_CACHE[key]


def kernel(**inputs):
    """Full (unsharded) inputs -> full [4, 2048, 1024] float32 output.

    Shards across the 8 NeuronCores as (batch x head-half), runs the Bass
    kernel SPMD, and gathers: out[b] = partial(core 2b) + partial(core 2b+1)
    + bias (row-sharded fc_out -> partial-sum reduction at gather time).
    """
    from concourse.bass_utils import run_bass_kernel_spmd

    inputs = {k: np.asarray(v) for k, v in inputs.items()}
    cfg = Cfg()
    nc = _get_nc()
    maps = shard_inputs(inputs, cfg, mybir.dt.bfloat16)
    res = run_bass_kernel_spmd(nc, maps, core_ids=list(range(8)), trace=False)
    return gather_outputs(res.results, inputs)


# revision 14
# speedup vs baseline: 1.0757x; 1.0757x over previous
"""Multi-head attention Bass/Tile kernel for TRN2, sharded 8 ways.

Sharding: core c handles batch b = c//2 and heads half = c%2 (8 of 16 heads).
Each core computes, for its batch and its 8 heads:
  q/k/v projections -> scoresT = K @ Q^T (per head, [t, s] layout) -> exp ->
  PV matmul with a ones-column appended to V (gives row sums for free) ->
  normalize -> partial output projection against its 512 rows of Wout^T.
Host sums the two partials per batch and adds the bias.

Layout choices (all chosen so NO transposes are needed anywhere):
  xT     [D, S]  : host-pretransposed activations (d on partitions)
  wq/wk  [D, H*dk] : lhsT layout for qT/kT = W^T @ xT
  wv     [D, H*dk] : rhs layout for v = xT^T @ wv  ([t, vdim], natural)
  kT     [H*dk, S]: j on partitions -> head-pair p lives in 128-row chunk p
                    (head A in rows 0:64, head B in rows 64:128)
  qT     [128, S_BLK] per (hp, sb): head A q-dims in rows 0:64, B in 64:128
  scoresT[t, s]   : ROW-TILED pair of K=64 matmuls — head A contracts on PE
                    array rows 0-63 (tile_position (0,0)), head B on rows
                    64-127 ((64,0), auto-derived from base_partition 64 of
                    kT[64:128]/qT[64:128]).  The two MMs run CONCURRENTLY
                    (different row groups, different psum banks) — ~2x
                    scores throughput vs a zero-padded K=128 formulation,
                    and no memsets/padded copies.
  ps tile [128, 1024] f32 (2 banks) = {A-chunk | B-chunk}: one exp
                    activation instr (N=1024) per t-chunk covers both heads.
  softmax sum over t folded into the PV matmul via the ones column of v'.
  out    [s, o]   : lhsT=concatT [i,s], rhs=woutT [i,o]

Scheduling: one software pipeline over units (sb, hp).  The ACT engine is
the bottleneck (~286us of exp work), so the prologue before the first
scores matmul is minimal (kT tb0/jc0 + qT0, 40 MMs): everything else (kT
remainder, v' projection, later qT blocks, out-projections) rides as
fillers inside the scores loops.  PV of unit k-1 trails unit k's scores
by 2 chunks.  Emission order is dependency-consistent per engine queue
(producer MMs always emitted before consumer MMs): kT jc0/jc1 complete
within unit 0's fillers, v' chunks complete ahead of their PV consumers
in unit 1, xk blocks stay resident (bufs=4) to avoid reload ordering.

HW pitfalls baked in (learned on-device):
  - no partition-shifting DVE copies (sim allows them, HW corrupts);
    the only cross-partition moves are InstReciprocal sbuf[64:65]->sbuf[0:1]
    (verified on HW) and gpsimd partition_broadcast
  - reciprocal_approx_fast (custom DVE op) produces garbage on HW
  - matmul free dim capped at 512 (fp32 psum); psum tiles bank-aligned
"""

from contextlib import ExitStack
from dataclasses import dataclass

import numpy as np
import ml_dtypes

import concourse.bass as bass  # noqa: F401
import concourse.tile as tile
from concourse import bacc, mybir


@dataclass
class Cfg:
    D: int = 1024      # model dim
    S: int = 2048      # sequence length (queries == keys)
    HL: int = 8        # heads per core
    DK: int = 64       # head dim
    S_BLK: int = 512   # query block (matmul free dim)
    T_BLK: int = 512   # t block in projection phase

    @property
    def DC(self):
        return self.D // 128

    @property
    def NSB(self):
        return self.S // self.S_BLK

    @property
    def TBn(self):
        return self.S // self.T_BLK

    @property
    def TCn(self):
        return self.S // 128

    @property
    def JW(self):
        return self.HL * self.DK

    @property
    def JC(self):
        return self.JW // 128

    @property
    def VW(self):
        return self.DK + 1

    @property
    def OB(self):
        return min(512, self.D)


DT_NP = {
    mybir.dt.bfloat16: ml_dtypes.bfloat16,
    mybir.dt.float32: np.float32,
    mybir.dt.float32r: np.float32,
}


def build_nc(cfg: Cfg, DT=mybir.dt.bfloat16, num_devices: int = 8):
    c = cfg
    f32 = mybir.dt.float32
    EXPDT = DT if DT == mybir.dt.bfloat16 else f32
    SCALE = 1.0 / float(np.sqrt(c.DK))
    nc = bacc.Bacc("TRN2", target_bir_lowering=False, debug=False,
                   num_devices=num_devices)

    xqT = nc.dram_tensor("xqT", [c.D, c.S], DT, kind="ExternalInput").ap()
    xkT = nc.dram_tensor("xkT", [c.D, c.S], DT, kind="ExternalInput").ap()
    xvT = nc.dram_tensor("xvT", [c.D, c.S], DT, kind="ExternalInput").ap()
    wq_d = nc.dram_tensor("wq", [c.D, c.JW], DT, kind="ExternalInput").ap()
    wk_d = nc.dram_tensor("wk", [c.D, c.JW], DT, kind="ExternalInput").ap()
    wv_d = nc.dram_tensor("wv", [c.D, c.JW], DT, kind="ExternalInput").ap()
    wo_d = nc.dram_tensor("woutT", [c.JW, c.D], DT, kind="ExternalInput").ap()
    out_d = nc.dram_tensor("out", [c.S, c.D], f32, kind="ExternalOutput").ap()

    with tile.TileContext(nc) as tc, ExitStack() as es:
        wpool = es.enter_context(tc.tile_pool(name="weights", bufs=1))
        kvpool = es.enter_context(tc.tile_pool(name="kv", bufs=1))
        xpool = es.enter_context(tc.tile_pool(name="x", bufs=2))
        qpool = es.enter_context(tc.tile_pool(name="q", bufs=2))
        epool = es.enter_context(tc.tile_pool(name="exp", bufs=2))
        cpool = es.enter_context(tc.tile_pool(name="cat", bufs=2))
        opool = es.enter_context(tc.tile_pool(name="o", bufs=2))
        rpool = es.enter_context(tc.tile_pool(name="r", bufs=1))
        pspool = es.enter_context(tc.tile_pool(name="ps", bufs=2, space="PSUM"))
        pvpool = es.enter_context(tc.tile_pool(name="pv", bufs=2, space="PSUM"))
        fppool = es.enter_context(tc.tile_pool(name="fp", bufs=2, space="PSUM"))
        stpool = es.enter_context(tc.tile_pool(name="st", bufs=2))

        def load_w_dmaj(dram, tag):
            t = wpool.tile([128, c.DC * c.JW], DT, tag=tag, name=tag)
            for d in range(c.DC):
                eng = nc.sync if d % 2 == 0 else nc.gpsimd
                eng.dma_start(t[:, d * c.JW:(d + 1) * c.JW],
                              dram[d * 128:(d + 1) * 128, :])
            return t

        def load_x_blk(dram, blk, name, tag="x", bufs=None):
            t = xpool.tile([128, c.DC * c.S_BLK], DT, tag=tag, name=name,
                           bufs=bufs)
            for d in range(c.DC):
                eng = nc.sync if d % 2 == 0 else nc.gpsimd
                eng.dma_start(
                    t[:, d * c.S_BLK:(d + 1) * c.S_BLK],
                    dram[d * 128:(d + 1) * 128,
                         blk * c.S_BLK:(blk + 1) * c.S_BLK])
            return t

        kT_sb = kvpool.tile([128, c.JC * c.S], DT)
        v_sb = kvpool.tile([128, c.TCn * c.HL * c.VW], DT)
        box = {}  # late-bound tiles: xk/xv/wv/wo

        # ---- leading weight + activation DMAs, interleaved round-robin
        # so the kT-jc0 and qT0 matmul chains both start early (the DMA
        # queue-op rate, ~650ns/op on 2 queues, paces the ramp).
        wk_sb = wpool.tile([128, c.DC * c.JW], DT, tag="wk", name="wk")
        wq_sb = wpool.tile([128, c.DC * c.JW], DT, tag="wq", name="wq")
        box["xk0"] = xpool.tile([128, c.DC * c.S_BLK], DT, tag="xk",
                                name="xk0", bufs=4)
        xq_tiles = {0: xpool.tile([128, c.DC * c.S_BLK], DT, tag="x",
                                  name="xq0")}
        for d in range(c.DC):
            r = slice(d * 128, (d + 1) * 128)
            w = slice(d * c.JW, (d + 1) * c.JW)
            s = slice(d * c.S_BLK, (d + 1) * c.S_BLK)
            nc.sync.dma_start(wk_sb[:, w], wk_d[r, :])
            nc.gpsimd.dma_start(box["xk0"][:, s], xkT[r, 0:c.S_BLK])
            nc.sync.dma_start(wq_sb[:, w], wq_d[r, :])
            nc.gpsimd.dma_start(xq_tiles[0][:, s], xqT[r, 0:c.S_BLK])

        def kt_ops(tb, jc):
            """8 accumulating MMs + 1 cast into kT (one jc, one t-block)."""
            ops = []
            pbox = {}

            def mk(d):
                def op():
                    if d == 0:
                        pbox[0] = fppool.tile([128, c.S_BLK], f32, tag="fp",
                                              name=f"psk{tb}_{jc}")
                    nc.tensor.matmul(
                        pbox[0][:],
                        wk_sb[:, d * c.JW + jc * 128: d * c.JW + (jc + 1) * 128],
                        box[f"xk{tb}"][:, d * c.S_BLK:(d + 1) * c.S_BLK],
                        start=(d == 0), stop=(d == c.DC - 1))
                    if d == c.DC - 1:
                        nc.vector.tensor_copy(
                            kT_sb[:, jc * c.S + tb * c.S_BLK:
                                  jc * c.S + (tb + 1) * c.S_BLK],
                            pbox[0][:])
                return op
            for d in range(c.DC):
                ops.append(mk(d))
            return ops

        def qT_ops(sb, qT):
            """Per jc: 8 accumulating MMs + 1 copy (A rows 0:64, B rows
            64:128 — natural psq layout, no zero-pad)."""
            ops = []
            pbox = {}

            def mk(jc, d):
                def op():
                    if d == 0:
                        pbox[jc] = fppool.tile([128, c.S_BLK], f32, tag="fp",
                                               name=f"psq{sb}_{jc}")
                    nc.tensor.matmul(
                        pbox[jc][:],
                        wq_sb[:, d * c.JW + jc * 128: d * c.JW + (jc + 1) * 128],
                        xq_tiles[sb][:, d * c.S_BLK:(d + 1) * c.S_BLK],
                        start=(d == 0), stop=(d == c.DC - 1))
                    if d == c.DC - 1:
                        nc.vector.tensor_copy(
                            qT[:, jc * c.S_BLK:(jc + 1) * c.S_BLK],
                            pbox[jc][:])
                return op
            for jc in range(c.JC):
                for d in range(c.DC):
                    ops.append(mk(jc, d))
            return ops

        def v_ops():
            """Per t-chunk g: 8 accumulating MMs + 1 strided copy into v'.
            xv2/xv3 loads woven in after their predecessors' consumers."""
            ops = []
            pbox = {}

            def mk(g, d):
                tb, tt = divmod(g, c.S_BLK // 128)

                def op():
                    if d == 0:
                        pbox[g] = fppool.tile([128, c.JW], f32, tag="fp",
                                              name=f"psv{g}")
                    nc.tensor.matmul(
                        pbox[g][:],
                        box[f"xv{tb}"][:, d * c.S_BLK + tt * 128:
                                       d * c.S_BLK + (tt + 1) * 128],
                        box["wv"][:, d * c.JW:(d + 1) * c.JW],
                        start=(d == 0), stop=(d == c.DC - 1))
                    if d == c.DC - 1:
                        dst = v_sb[:, g * c.HL * c.VW:(g + 1) * c.HL * c.VW]
                        dst3 = dst.rearrange("p (h w) -> p h w",
                                             w=c.VW)[:, :, 0:c.DK]
                        src3 = pbox[g][:].rearrange("p (h w) -> p h w", w=c.DK)
                        nc.vector.tensor_copy(dst3, src3)
                        del pbox[g]
                return op

            def ld(tb):
                def op():
                    box[f"xv{tb}"] = load_x_blk(xvT, tb, f"xv{tb}")
                return op
            for g in range(c.TCn):
                for d in range(c.DC):
                    ops.append(mk(g, d))
                if g == 3:
                    ops.append(ld(2))
                if g == 7:
                    ops.append(ld(3))
            return ops

        def outproj_ops(sb, catT):
            """Per (sc, oc): 4 ic-MMs into a 1-bank psum, then copy + DMA."""
            ops = []
            po_box = {}

            def mk(sc, oc, ic):
                def op():
                    if ic == 0:
                        po_box[(sc, oc)] = fppool.tile(
                            [128, c.OB], f32, tag="fp", name=f"po{sb}_{sc}_{oc}")
                    po = po_box[(sc, oc)]
                    nc.tensor.matmul(
                        po[:],
                        catT[:, ic * c.S_BLK + sc * 128:
                             ic * c.S_BLK + (sc + 1) * 128],
                        box["wo"][:, ic * c.D + oc * c.OB:
                                  ic * c.D + (oc + 1) * c.OB],
                        start=(ic == 0), stop=(ic == c.JC - 1))
                    if ic == c.JC - 1:
                        ot = opool.tile([128, c.OB], f32, tag="ot",
                                        name=f"ot{sb}_{sc}_{oc}")
                        nc.vector.tensor_copy(ot[:], po[:])
                        eng = nc.sync if (2 * sc + oc) % 2 == 0 else nc.gpsimd
                        eng.dma_start(
                            out_d[sb * c.S_BLK + sc * 128:
                                  sb * c.S_BLK + (sc + 1) * 128,
                                  oc * c.OB:(oc + 1) * c.OB],
                            ot[:])
                return op
            for sc in range(c.S_BLK // 128):
                for oc in range(c.D // c.OB):
                    for ic in range(c.JC):
                        ops.append(mk(sc, oc, ic))
            return ops

        def emit_pv_chunk(u, t0, nt):
            eb = u["exp"]
            for t in range(t0, t0 + nt):
                nc.tensor.matmul(
                    u["pvA"][0:c.VW, :],
                    v_sb[:, t * c.HL * c.VW + (2 * u["hp"]) * c.VW:
                         t * c.HL * c.VW + (2 * u["hp"] + 1) * c.VW],
                    eb[:, t * 1024: t * 1024 + c.S_BLK],
                    start=(t == 0), stop=(t == c.TCn - 1))
                nc.tensor.matmul(
                    u["pvB"][0:c.VW, :],
                    v_sb[:, t * c.HL * c.VW + (2 * u["hp"] + 1) * c.VW:
                         t * c.HL * c.VW + (2 * u["hp"] + 2) * c.VW],
                    eb[:, t * 1024 + c.S_BLK:(t + 1) * 1024],
                    start=(t == 0), stop=(t == c.TCn - 1))

        def emit_stage(u):
            sb, hp = u["sb"], u["hp"]
            u["stA"] = stpool.tile([c.VW, c.S_BLK], f32, tag="stA",
                                   name=f"stA{sb}_{hp}")
            u["stB"] = stpool.tile([c.VW, c.S_BLK], f32, tag="stB",
                                   name=f"stB{sb}_{hp}")
            nc.vector.tensor_copy(u["stA"][:], u["pvA"][0:c.VW, :])
            nc.vector.tensor_copy(u["stB"][:], u["pvB"][0:c.VW, :])

        def emit_normalize(u):
            sb, hp = u["sb"], u["hp"]
            stA, stB, catT = u["stA"], u["stB"], u["catT"]
            rtiA = rpool.tile([1, c.S_BLK], f32, tag="rtiA", name=f"rtiA{sb}_{hp}")
            rtiB = rpool.tile([1, c.S_BLK], f32, tag="rtiB", name=f"rtiB{sb}_{hp}")
            # NB: cross-partition (row 64 -> row 0) — verified OK on HW for
            # InstReciprocal specifically.
            nc.vector.reciprocal(rtiA[:], stA[c.DK:c.DK + 1, :])
            nc.vector.reciprocal(rtiB[:], stB[c.DK:c.DK + 1, :])
            rbA = rpool.tile([c.DK, c.S_BLK], f32, tag="rbA", name=f"rbA{sb}_{hp}")
            rbB = rpool.tile([c.DK, c.S_BLK], f32, tag="rbB", name=f"rbB{sb}_{hp}")
            nc.gpsimd.partition_broadcast(rbA[:], rtiA[:])
            nc.gpsimd.partition_broadcast(rbB[:], rtiB[:])
            nc.vector.tensor_mul(
                catT[0:c.DK, hp * c.S_BLK:(hp + 1) * c.S_BLK],
                stA[0:c.DK, :], rbA[:])
            nc.vector.tensor_mul(
                catT[64:64 + c.DK, hp * c.S_BLK:(hp + 1) * c.S_BLK],
                stB[0:c.DK, :], rbB[:])

        # ---- minimal prologue: kT(tb0, jc0) + qT0 ----
        for op in kt_ops(0, 0):
            op()
        qT_tiles = {0: qpool.tile([128, c.JC * c.S_BLK], DT, tag="qT",
                                  name="qT0")}
        for op in qT_ops(0, qT_tiles[0]):
            op()

        # ---- filler / load schedule ----
        units = [(sb, hp) for sb in range(c.NSB) for hp in range(c.JC)]
        fillers = [[] for _ in units]
        loads = [[] for _ in units]

        def mk_ld(key, fn):
            def op():
                box[key] = fn()
            return op

        # unit 0 loads: remaining xk blocks (resident, bufs=4), wv, first
        # xv blocks; the v' ones-columns memset rides the gpsimd queue last.
        loads[0] += [
            mk_ld("xk1", lambda: load_x_blk(xkT, 1, "xk1", tag="xk", bufs=4)),
            mk_ld("xk2", lambda: load_x_blk(xkT, 2, "xk2", tag="xk", bufs=4)),
            mk_ld("xk3", lambda: load_x_blk(xkT, 3, "xk3", tag="xk", bufs=4)),
            mk_ld("wv", lambda: load_w_dmaj(wv_d, "wv")),
            mk_ld("xv0", lambda: load_x_blk(xvT, 0, "xv0")),
            mk_ld("xv1", lambda: load_x_blk(xvT, 1, "xv1")),
            lambda: nc.gpsimd.memset(v_sb[:], 1.0),
        ]
        def load_wo():
            # wo shares wk's buffer (tag "wk"): its DMA waits on the kT MMs.
            t = wpool.tile([128, c.JC * c.D], DT, tag="wk", name="wo")
            for ic in range(c.JC):
                eng = nc.sync if ic % 2 == 0 else nc.gpsimd
                eng.dma_start(t[:, ic * c.D:(ic + 1) * c.D],
                              wo_d[ic * 128:(ic + 1) * 128, :])
            return t

        loads[1] += [mk_ld("xq1", lambda: load_x_blk(xqT, 1, "xq1"))]
        loads[2] += [mk_ld("wo", load_wo)]

        # unit 0 fillers: finish kT jc0 (scores deadline th4/8/12) and all
        # of jc1 (unit 1's scores), plus tb0's jc2/jc3.
        for tb in [1, 2, 3]:
            fillers[0] += kt_ops(tb, 0)
        for tb in [0, 1, 2, 3]:
            fillers[0] += kt_ops(tb, 1)
        fillers[0] += kt_ops(0, 2)
        fillers[0] += kt_ops(0, 3)          # 72 ops
        # unit 1 fillers: full v' stream (ahead of its PV consumers), then
        # the kT remainder (jc2 by unit 2, jc3 by unit 3).
        fillers[1] += v_ops()               # 130 ops incl 2 loads
        for tb in [1, 2, 3]:
            fillers[1] += kt_ops(tb, 2)
        for tb in [1, 2, 3]:
            fillers[1] += kt_ops(tb, 3)     # +48

        # ---- the main (sb, hp) software pipeline ----
        prev = None
        cat_tiles = {}

        for idx, (sb, hp) in enumerate(units):
            if hp == 0:
                cat_tiles[sb] = cpool.tile([128, c.JC * c.S_BLK], DT, tag="cat",
                                           name=f"catT{sb}")
            for ld in loads[idx]:
                ld()
            if hp == 2 and sb + 2 < c.NSB:
                # prefetch xq for the sb-after-next (qT rides (sb+1,0..1))
                xq_tiles[sb + 2] = load_x_blk(xqT, sb + 2, f"xq{sb + 2}")
            if (sb + 1 < c.NSB and hp == 2 and sb == 0) or \
                    (sb + 1 < c.NSB and hp == 0 and sb >= 1):
                # qT projection for the next sb: units (0,2)-(0,3) for sb1
                # (units 0-1 carry kT/v), else (sb,0)-(sb,1)
                if sb == 0:
                    xq_tiles[1] = box["xq1"]
                qT_tiles[sb + 1] = qpool.tile([128, c.JC * c.S_BLK], DT,
                                              tag="qT", name=f"qT{sb + 1}")
                qops = qT_ops(sb + 1, qT_tiles[sb + 1])
                fillers[idx] += qops[:16]
                fillers[idx + 1] += qops[16:]

            catT = cat_tiles[sb]
            qT = qT_tiles[sb]
            cur = {
                "sb": sb, "hp": hp, "catT": catT,
                "exp": epool.tile([128, c.TCn * 1024], EXPDT, tag="exp",
                                  name=f"exp{sb}_{hp}"),
            }
            if prev is not None:
                prev["pvA"] = pvpool.tile([c.VW, c.S_BLK], f32, tag="pv",
                                          name=f"pvA{prev['sb']}_{prev['hp']}")
                prev["pvB"] = pvpool.tile([c.VW, c.S_BLK], f32, tag="pv",
                                          name=f"pvB{prev['sb']}_{prev['hp']}")
            flist = fillers[idx]
            fpos = 0
            # Light-filler units run prev's PV at 2 chunks/th (th2..9) and
            # normalize at th9 — the reciprocal chain then overlaps the
            # unit's second half instead of piling up at unit boundaries.
            # Heavy prologue units (0-1) keep the 1-chunk trail so the v'
            # filler stream stays ahead of its PV consumers.
            front = prev is not None and idx == len(units) - 1
            for th in range(c.TCn):
                ps = pspool.tile([128, 1024], f32, tag="ps",
                                 name=f"ps{sb}_{hp}_{th}")
                lhsA = kT_sb[0:64, hp * c.S + th * 128: hp * c.S + (th + 1) * 128]
                lhsB = kT_sb[64:128, hp * c.S + th * 128: hp * c.S + (th + 1) * 128]
                # row-tiled concurrent pair: head A on PE rows 0-63,
                # head B on rows 64-127 (tile_position from base_partition)
                nc.tensor.matmul(
                    ps[:, 0:c.S_BLK], lhsA,
                    qT[0:64, hp * c.S_BLK:(hp + 1) * c.S_BLK],
                    start=True, stop=True)
                nc.tensor.matmul(
                    ps[:, c.S_BLK:1024], lhsB,
                    qT[64:128, hp * c.S_BLK:(hp + 1) * c.S_BLK],
                    start=True, stop=True)
                nc.scalar.activation(
                    cur["exp"][:, th * 1024:(th + 1) * 1024],
                    ps[:], mybir.ActivationFunctionType.Exp, scale=SCALE)
                if front:
                    if 2 <= th <= 9:
                        emit_pv_chunk(prev, 2 * (th - 2), 2)
                    if th == 9:
                        emit_stage(prev)
                        emit_normalize(prev)
                elif prev is not None:
                    if th >= 2:
                        emit_pv_chunk(prev, th - 2, 1)
                    if th == c.TCn - 1:
                        emit_pv_chunk(prev, c.TCn - 2, 2)
                        emit_stage(prev)
                # fillers: back-loaded (th8-15) when PV is front-loaded,
                # else spread evenly
                if front:
                    want = (len(flist) * max(0, th - 7)) // (c.TCn - 8)
                else:
                    want = (len(flist) * (th + 1)) // c.TCn
                while fpos < want:
                    flist[fpos]()
                    fpos += 1
            while fpos < len(flist):
                flist[fpos]()
                fpos += 1
            if prev is not None and not front:
                emit_normalize(prev)
            if prev is not None and prev["hp"] == c.JC - 1:
                # out-projection of prev's sb rides units (sb+1,2)-(sb+1,3);
                # the last sb's goes fully on (3,2) so unit 15 stays clean
                oops = outproj_ops(prev["sb"], prev["catT"])
                base = idx + 2
                if base + 1 < len(units) - 1:
                    fillers[base] += oops[:16]
                    fillers[base + 1] += oops[16:]
                else:
                    fillers[base] += oops
            prev = cur
        # ---- drain: split the last unit's PV by head so recip-A overlaps
        # PV-B on the PE; the out-projection's ic=0..2 matmuls have no
        # dependency on the last normalize (subtile deps) and fill the PE
        # during the reciprocal/broadcast chain.
        u = prev
        sb, hp = u["sb"], u["hp"]
        u["pvA"] = pvpool.tile([c.VW, c.S_BLK], f32, tag="pv", name="pvA_last")
        u["pvB"] = pvpool.tile([c.VW, c.S_BLK], f32, tag="pv", name="pvB_last")
        eb = u["exp"]
        for t in range(c.TCn):
            nc.tensor.matmul(
                u["pvA"][0:c.VW, :],
                v_sb[:, t * c.HL * c.VW + (2 * hp) * c.VW:
                     t * c.HL * c.VW + (2 * hp + 1) * c.VW],
                eb[:, t * 1024: t * 1024 + c.S_BLK],
                start=(t == 0), stop=(t == c.TCn - 1))
        u["stA"] = stpool.tile([c.VW, c.S_BLK], f32, tag="stA", name="stA_last")
        nc.vector.tensor_copy(u["stA"][:], u["pvA"][0:c.VW, :])
        rtiA = rpool.tile([1, c.S_BLK], f32, tag="rtiA", name="rtiA_last")
        nc.vector.reciprocal(rtiA[:], u["stA"][c.DK:c.DK + 1, :])
        rbA = rpool.tile([c.DK, c.S_BLK], f32, tag="rbA", name="rbA_last")
        nc.gpsimd.partition_broadcast(rbA[:], rtiA[:])
        for t in range(c.TCn):
            nc.tensor.matmul(
                u["pvB"][0:c.VW, :],
                v_sb[:, t * c.HL * c.VW + (2 * hp + 1) * c.VW:
                     t * c.HL * c.VW + (2 * hp + 2) * c.VW],
                eb[:, t * 1024 + c.S_BLK:(t + 1) * 1024],
                start=(t == 0), stop=(t == c.TCn - 1))
        u["stB"] = stpool.tile([c.VW, c.S_BLK], f32, tag="stB", name="stB_last")
        nc.vector.tensor_copy(u["stB"][:], u["pvB"][0:c.VW, :])
        rtiB = rpool.tile([1, c.S_BLK], f32, tag="rtiB", name="rtiB_last")
        nc.vector.reciprocal(rtiB[:], u["stB"][c.DK:c.DK + 1, :])
        rbB = rpool.tile([c.DK, c.S_BLK], f32, tag="rbB", name="rbB_last")
        nc.gpsimd.partition_broadcast(rbB[:], rtiB[:])
        nc.vector.tensor_mul(
            u["catT"][0:c.DK, hp * c.S_BLK:(hp + 1) * c.S_BLK],
            u["stA"][0:c.DK, :], rbA[:])
        nc.vector.tensor_mul(
            u["catT"][64:64 + c.DK, hp * c.S_BLK:(hp + 1) * c.S_BLK],
            u["stB"][0:c.DK, :], rbB[:])
        for op in outproj_ops(sb, u["catT"]):
            op()

    nc.compile()
    return nc


def shard_inputs(inputs: dict, cfg: Cfg, DT=mybir.dt.bfloat16):
    """Full inputs -> list of 8 per-core in_maps (numpy)."""
    npdt = DT_NP[DT]
    q, k, v = inputs["queries"], inputs["keys"], inputs["values"]
    Wq, Wk, Wv = inputs["Wq"], inputs["Wk"], inputs["Wv"]
    Wout = inputs["Wout"]
    B = q.shape[0]
    maps = []
    WoutT = np.ascontiguousarray(Wout.T)  # [i, o]
    for core in range(2 * B):
        b, half = divmod(core, 2)
        hs = slice(half * cfg.HL, (half + 1) * cfg.HL)
        i0 = half * cfg.JW
        maps.append({
            "xqT": np.ascontiguousarray(q[b].T).astype(npdt),
            "xkT": np.ascontiguousarray(k[b].T).astype(npdt),
            "xvT": np.ascontiguousarray(v[b].T).astype(npdt),
            "wq": np.ascontiguousarray(
                Wq[hs].transpose(1, 0, 2).reshape(cfg.D, cfg.JW)).astype(npdt),
            "wk": np.ascontiguousarray(
                Wk[hs].transpose(1, 0, 2).reshape(cfg.D, cfg.JW)).astype(npdt),
            "wv": np.ascontiguousarray(
                Wv[hs].transpose(1, 0, 2).reshape(cfg.D, cfg.JW)).astype(npdt),
            "woutT": np.ascontiguousarray(WoutT[i0:i0 + cfg.JW]).astype(npdt),
        })
    return maps


def gather_outputs(results, inputs):
    bout = inputs["bout"]
    B = inputs["queries"].shape[0]
    outs = []
    for b in range(B):
        outs.append(results[2 * b]["out"] + results[2 * b + 1]["out"] + bout)
    return np.stack(outs).astype(np.float32)


def percore_reference(in_map: dict, cfg: Cfg):
    """Numpy reference of what one core should produce (fp32 math)."""
    c = cfg
    xq = in_map["xqT"].astype(np.float32).T   # [S, D]
    xk = in_map["xkT"].astype(np.float32).T
    xv = in_map["xvT"].astype(np.float32).T
    wq = in_map["wq"].astype(np.float32)      # [D, JW]
    wk = in_map["wk"].astype(np.float32)
    wv = in_map["wv"].astype(np.float32)
    wo = in_map["woutT"].astype(np.float32)   # [JW, D]
    q = xq @ wq                               # [S, JW]
    k = xk @ wk
    v = xv @ wv
    cat = np.zeros((c.S, c.JW), dtype=np.float32)
    for h in range(c.HL):
        sl = slice(h * c.DK, (h + 1) * c.DK)
        s = (q[:, sl] @ k[:, sl].T) / np.sqrt(c.DK)
        e = np.exp(s)
        p = e / e.sum(axis=1, keepdims=True)
        cat[:, sl] = p @ v[:, sl]
    return cat @ wo

# ----------------------------------------------------------------------------
# Self-contained entry point: kernel(**inputs) -> full [B, S, D] output.
# ----------------------------------------------------------------------------
_NC_CACHE = {}


def _get_nc():
    key = "attn"
    if key not in _NC_CACHE:
        _NC_CACHE[key] = build_nc(Cfg(), mybir.dt.bfloat16, num_devices=8)
    return _NC_CACHE[key]


def kernel(**inputs):
    """Full (unsharded) inputs -> full [4, 2048, 1024] float32 output.

    Shards across the 8 NeuronCores as (batch x head-half), runs the Bass
    kernel SPMD, and gathers: out[b] = partial(core 2b) + partial(core 2b+1)
    + bias (row-sharded fc_out -> partial-sum reduction at gather time).
    """
    from concourse.bass_utils import run_bass_kernel_spmd

    inputs = {k: np.asarray(v) for k, v in inputs.items()}
    cfg = Cfg()
    nc = _get_nc()
    maps = shard_inputs(inputs, cfg, mybir.dt.bfloat16)
    res = run_bass_kernel_spmd(nc, maps, core_ids=list(range(8)), trace=False)
    return gather_outputs(res.results, inputs)
